# revision 55
# baseline (speedup 1.0000x reference)
"""Two-layer GAT forward on 8 Trainium2 NeuronCores (HW ~650us).

Strategy: edges partitioned by destination across the 8 cores (1250 dsts
per core).  Each core's local nodes are PERMUTED (in-degree-balanced LPT
dealing + swap repair) into 40 quarter-bins per device — 10 windows of
128 dsts x 4 quarters of 32 — so most quarters need exactly 4 edge
chunks of 128 (<=512 edges); overflow is rotated into quarter 0.  Node
features are replicated for the layer-1 GEMM (bf16), whose output rows
are stored in fp8(e3m4) with bf16 attention scores packed in the same
1280B row, in per-device rolled+permuted order (windows align with GEMM
tiles; dst scores come straight from the GEMM epilogue).

Per-edge source rows are fetched with one dma_gather per 5-chunk group
(HARD LIMIT: >640 indices per dma_gather hangs the device), round-robin
over 4 SWDGE queues (num_swdge_queues=4; queue index must stay in
lockstep with Tile's global 8-lane DMASW sem rotation or the sim flags
queue/sem locking).  The aggregation runs as per-(chunk, head) matmuls:
alpha rides a 32-wide quarter-relative one-hot lhsT (bf16) against the
RAW gathered fp8 rows as rhs (mixed-dtype matmul), accumulating into a
[128,1024] psum window at 32-aligned tile_position partition offsets;
denominators use the unweighted one-hot vs pb.  PSUM small tiles are
padded to full 2KB banks — pools pack tags into shared banks and a
shared "zero region" with an open accumulation group poisons reads of
neighbours.  Only one start/stop per (bank x partition-range): start
marks the whole 2KB region pending-zero, later head blocks overwrite
their bytes on first touch.  The layer-2 GEMM is fused into the window
epilogue; the AllGather of layer-2 rows is split in two so the first
half overlaps windows 5-9 (collectives MUST sit between pool scopes).
Layer-2 aggregation mirrors layer 1 (1 head, 512B bf16 rows).

Measured facts / dead-ends (do not retry blindly):
- dma_gather engine-hold ~1us + ~7ns/index (desc-gen on the Pool
  engine); transfers run async on 16 DMA engines (~81ns/1280B row per
  engine).  Multi-queue only overlaps transfers, not desc-gen: ~5.5us
  hold per 640-idx gather is the floor here.  68 gathers total.
- dynamic_dma_scratch_size=32768 did NOT shrink ring-reuse holds.
- fp8e4 DoubleRow GEMM for phase A FAILS accuracy (3.6e-2): random-sign
  dot products keep the 6% quantization error relative to the sum.
- prepare_only async gathers unsound under this Tile build
  (_fix_prep_sems has the partial lane rewiring, still needs injected
  wait-chains); src-sorted gathers regress; deeper buffering neutral.

Self-contained: hardcodes the problem shapes from the spec.
"""
import os
import sys
import numpy as np

try:
    import concourse.bass as bass  # noqa
except ImportError:
    sys.path.insert(0, "/opt/trn_rl_repo")

import concourse.bass as bass
import concourse.tile as tile
from concourse import mybir, bacc
from concourse.bass_utils import run_bass_kernel_spmd

# ---------------------------------------------------------------- problem dims
N, E = 10000, 160000
FIN, H1, C1, C2 = 256, 8, 128, 128
D1 = H1 * C1  # 1024
SLOPE = 0.2
NDEV = 8
NLOC = N // NDEV  # 1250
NW = (NLOC + 127) // 128  # 10 windows of 128 dsts (last = 98)
LASTW = NLOC - (NW - 1) * 128  # 98
NT = (N + 127) // 128  # 79 node tiles for the replicated GEMM
SPLIT = 640  # windows 0-4 -> first AllGather
REST = NLOC - SPLIT  # 610

F32 = mybir.dt.float32
BF16 = mybir.dt.bfloat16
FP8 = mybir.dt.float8e3
FP8E4 = mybir.dt.float8e4
I16 = mybir.dt.int16
ALU = mybir.AluOpType
ACTF = mybir.ActivationFunctionType

# hcat row (fp8 bytes): [ h 1024 | s_src 8xbf16 (bf16 slots 512:520) | pad ]
ROW1 = 1280
# layer-2 row (bf16): [ h2 128 | s2src f32 (f32 slot 64) | pad ] -> 256 = 512B
ROW2 = 256

_EPS = 1e-30

# --------------------------------------------------------------------- patches


def _apply_drain_patch():
    """This walrus build rejects >1 sync-wait on the Tile-exit Drain; split the
    waits across consecutive drains (semantically identical)."""
    from concourse.vector_clock import ScopedClock

    def _patched(self, tick_clock, wait_clock):
        drain_inst = self.nc.sync.drain()
        wait_clock.add_sem_waits(
            drain_inst.ins, ScopedClock({None: tick_clock.global_clock})
        )
        si = drain_inst.ins.sync_info
        if si is not None and len(si.on_wait) > 1:
            waits = list(si.on_wait)
            si.on_wait = waits[:1]
            drain_inst.ins.sync_info = si
            for i in range(1, len(waits)):
                extra = self.nc.sync.drain()
                esi = extra.ins.sync_info
                if esi is None:
                    esi = mybir.SyncInfo(on_wait=[], on_update=[])
                esi.on_wait = list(esi.on_wait) + waits[i : i + 1]
                extra.ins.sync_info = esi
        self.nc.all_engine_barrier()
        assert self.sems is not None
        popped = self.nc._tile_sem_poison_stack.pop()
        assert popped is self._sem_poison
        self.nc.clear_and_free_semaphores(list(self.sems.allocated().values()))
        self.nc.all_engine_barrier()

    tile.TileContext._drain_and_barrier = _patched


_apply_drain_patch()


def _fix_prep_sems(nc):
    """PREPARE_ONLY SWDGE preps bake the user-supplied sem into their DMA
    descriptors, but Tile's wait pass makes data consumers wait on the DMASW
    lane sems it assigned (assuming the descriptors bump those).  Re-point
    each prep's descriptor sem at its assigned DMASW lane sem (exact lane
    from bass_scheduled_proc, scope suffix from the Pool engine-sem entry).
    Per-engine in-order ring draining makes the lane-merged waits sound."""
    from concourse.tile_sem_assignment import PROC_NAME_TO_IDX

    idx2name = {v: k for k, v in PROC_NAME_TO_IDX.items()}
    fn = nc.m.functions[0]
    name2id = {}
    for b in fn.blocks:
        for ins in b.instructions:
            si = ins.sync_info
            if si is None:
                continue
            for x in list(si.on_wait) + list(si.on_update):
                if getattr(x, "ant_name", None):
                    name2id[x.ant_name] = x.id
    nfix = 0
    for b in fn.blocks:
        for ins in b.instructions:
            if type(ins).__name__ != "InstDMAGatherAnt":
                continue
            if getattr(ins, "gen_mode", 0) != 1:
                continue
            lane = idx2name[ins.bass_scheduled_proc]
            assert lane.startswith("DMASW"), lane
            si = ins.sync_info
            ups = list(si.on_update)
            scope = None
            for x in ups:
                if x.ant_name.startswith("Pool_"):
                    scope = x.ant_name.split("_", 1)[1]
            assert scope is not None, ups
            target = f"{lane}_{scope}"
            assert target in name2id, (target, sorted(name2id))
            hit = False
            for x in ups:
                if x.ant_name == "gsem":
                    x.id = name2id[target]
                    x.ant_name = target
                    hit = True
            assert hit, ups
            si.on_update = ups
            ins.sync_info = si
            nfix += 1
    assert nfix > 0
    # Tile's one-wait-per-instruction merge can under-synchronize async
    # preps.  Conservatively raise every DMASW wait to the lane's cumulative
    # count as of the last earlier prep in scheduled order (triggers are
    # unconditional, so raised waits always satisfy eventually).
    cum = {}
    for b in fn.blocks:
        for ins in b.instructions:
            si = ins.sync_info
            if si is not None and si.on_wait:
                ws = list(si.on_wait)
                changed = False
                for x in ws:
                    c = cum.get(x.ant_name)
                    if (
                        x.ant_name.startswith("DMASW")
                        and c is not None
                        and x.wait_value is not None
                        and c > x.wait_value
                    ):
                        x.wait_value = c
                        changed = True
                if changed:
                    si.on_wait = ws
                    ins.sync_info = si
            if (
                type(ins).__name__ == "InstDMAGatherAnt"
                and getattr(ins, "gen_mode", 0) == 1
            ):
                upd = ins.sync_info.on_update[0]
                assert upd.ant_name.startswith("DMASW"), upd
                cum[upd.ant_name] = cum.get(upd.ant_name, 0) + upd.update_value
    return nfix


# ------------------------------------------------------------------- host prep


def _wrap_idx(idx):
    """dma_gather index layout: idx i at partition i%16, col i//16, replicated
    8x across the 128 partitions."""
    a = np.ascontiguousarray(idx.astype(np.int16).reshape(-1, 16).T)
    return np.ascontiguousarray(np.tile(a, (8, 1)))


def _bf(a):
    import ml_dtypes

    return np.ascontiguousarray(a).astype(ml_dtypes.bfloat16)


def _quarter_caps(w):
    """Quarter q of window w covers local positions
    [w*128 + q*32, min(w*128 + (q+1)*32, NLOC)); returns the 4 sizes."""
    caps = []
    for q in range(4):
        lo = w * 128 + q * 32
        hi = min(w * 128 + (q + 1) * 32, NLOC)
        caps.append(max(0, hi - lo))
    return caps


def _prep_edges(edge_index):
    """Shard edges by dst across devices.  Each device's 1250 local nodes
    are PERMUTED (in-degree LPT dealing) into 40 quarter-bins (10 windows
    x 4 quarters of 32 dsts) so per-quarter edge counts are balanced; each
    quarter's edges are padded to Q[w][q]*128 slots (Q = max over devices,
    >=1).  One dma_gather per (window, quarter), <=640 indices.  Self-loop
    edges are handled exactly on-device from local activations."""
    src = np.asarray(edge_index[0], np.int64)
    dst = np.asarray(edge_index[1], np.int64)

    # per-device balanced quarter assignment
    dev_edges = []  # (s_d, t_d)
    dev_pos = []  # pos_local[node-id-offset] -> physical position
    dev_qedges = []  # per (w,q): (srcs, dst_positions)
    Q = np.ones((NW, 4), np.int64)
    for d in range(NDEV):
        base = d * NLOC
        sel = (dst >= base) & (dst < base + NLOC)
        s_d, t_d = src[sel], dst[sel] - base
        deg = np.bincount(t_d, minlength=NLOC)
        caps = [[_quarter_caps(w)[q] for q in range(4)] for w in range(NW)]
        bins = [
            (w, q) for w in range(NW) for q in range(4) if caps[w][q] > 0
        ]
        load = {b: 0 for b in bins}
        room = {b: caps[b[0]][b[1]] for b in bins}
        binidx = {b: i for i, b in enumerate(bins)}
        members = [[] for _ in bins]
        for v in np.argsort(-deg, kind="stable"):
            best = min((b for b in bins if room[b] > 0), key=lambda b: load[b])
            load[best] += int(deg[v])
            room[best] -= 1
            members[binidx[best]].append(int(v))
        # swap-repair: each window designates an overflow bin that absorbs
        # the load above 512/bin; the others are swapped down to <=512
        # (= one 128-chunk fewer).  The overflow bin is then rotated into
        # quarter 0 so residual Q=5 quarters line up across devices.
        LIM, GCAP = 512, 640

        def _swap_down(b_from, b_to, lim_from):
            """Swap node pairs to bring load[b_from] <= lim_from, raising
            load[b_to].  Returns True on success."""
            for _ in range(40):
                need = load[b_from] - lim_from
                if need <= 0:
                    return True
                best = None
                for a in members[binidx[b_from]]:
                    for c in members[binidx[b_to]]:
                        dl = int(deg[a]) - int(deg[c])
                        if dl <= 0:
                            continue
                        score = (dl >= need, -dl if dl >= need else dl)
                        if best is None or score > best[0]:
                            best = (score, a, c)
                if best is None:
                    return False
                _, a, c = best
                members[binidx[b_from]].remove(a)
                members[binidx[b_to]].remove(c)
                members[binidx[b_from]].append(c)
                members[binidx[b_to]].append(a)
                load[b_from] -= int(deg[a]) - int(deg[c])
                load[b_to] += int(deg[a]) - int(deg[c])
            return load[b_from] <= lim_from

        for w in range(NW):
            wb32 = [b for b in bins if b[0] == w and caps[b[0]][b[1]] == 32]
            sac = max(wb32, key=lambda b: load[b])
            for b in wb32:
                if b != sac and load[b] > LIM:
                    _swap_down(b, sac, LIM)
            for b in wb32:  # spill overflow past the gather cap sideways
                if load[b] > GCAP:
                    for b2 in sorted(wb32, key=lambda x: load[x]):
                        if b2 != b and _swap_down(b, b2, GCAP):
                            break
            assert all(load[b] <= GCAP for b in wb32), (w, [load[b] for b in wb32])
            # rotate heaviest-first among the 32-cap bins
            order = sorted(wb32, key=lambda b: -load[b])
            oldm = [members[binidx[b]] for b in order]
            oldl = [load[b] for b in order]
            for b, m, ld in zip(sorted(wb32), oldm, oldl):
                members[binidx[b]] = m
                load[b] = ld
        # physical order: bins in (w,q) order, nodes sorted by id inside
        pos = np.empty(NLOC, np.int64)
        p = 0
        qe = {}
        for i, b in enumerate(bins):
            mem = sorted(members[i])
            for v in mem:
                pos[v] = p
                p += 1
        assert p == NLOC
        # per-quarter edge lists (by dst physical position)
        tpos = pos[t_d]
        wq = (tpos // 128) * 4 + (tpos % 128) // 32
        order = np.argsort(tpos, kind="stable")
        s_s, tp_s, wq_s = s_d[order], tpos[order], wq[order]
        for w in range(NW):
            for q in range(4):
                m = wq_s == w * 4 + q
                qe[(w, q)] = (s_s[m], tp_s[m])
                Q[w, q] = max(Q[w, q], (m.sum() + 127) // 128)
        dev_pos.append(pos)
        dev_qedges.append(qe)

    QTAB = [[int(Q[w, q]) for q in range(4)] for w in range(NW)]
    assert max(max(r) for r in QTAB) <= 5, QTAB  # 640-idx gather cap
    NCHUNK = int(Q.sum())
    TOT = NCHUNK * 128

    # global physical node order (ids), per device d the hcat order is
    # roll(order_ids, -d*NLOC)
    order_ids = np.concatenate(
        [d * NLOC + np.argsort(dev_pos[d], kind="stable") for d in range(NDEV)]
    )
    gpos = np.empty(N, np.int64)
    gpos[order_ids] = np.arange(N)

    devs = []
    for d in range(NDEV):
        srcp = np.zeros(TOT, np.int64)
        dloc32 = np.full(TOT, -1.0, np.float32)  # quarter-relative
        dlocw = np.full(TOT, -1.0, np.float32)  # window-relative
        o = 0
        for w in range(NW):
            for q in range(4):
                s_q, tp_q = dev_qedges[d][(w, q)]
                n = len(s_q)
                srcp[o : o + n] = s_q
                dloc32[o : o + n] = (tp_q - w * 128 - q * 32).astype(
                    np.float32
                )
                dlocw[o : o + n] = (tp_q - w * 128).astype(np.float32)
                o += QTAB[w][q] * 128
        assert o == TOT
        # layer-1 gather rows in device-d hcat order
        src1 = (gpos[srcp] - d * NLOC) % N
        # layer-2 gather rows: h2all is [AG1: 8xSPLIT | AG2: 8xREST],
        # indexed by (owner r, owner-local physical position l)
        r = srcp // NLOC
        l = gpos[srcp] - r * NLOC
        if AGSPLIT:
            src2 = np.where(
                l < SPLIT, r * SPLIT + l, NDEV * SPLIT + r * REST + (l - SPLIT)
            )
        else:
            src2 = r * NLOC + l
        devs.append(
            {
                "srcidx": _wrap_idx(src1),
                "srcidx2": _wrap_idx(src2),
                "dstloc": _bf(dloc32.reshape(NCHUNK, 128).T),
                "dstrep": _bf(np.tile(dlocw[None, :], (128, 1))),
            }
        )
    return devs, QTAB, order_ids


# -------------------------------------------------------------- program build

_CACHE = {}

AGSPLIT = os.environ.get("KAGSPLIT", "1") == "1"


QMAX5 = 5  # chunk slots per quarter tile (640-idx gather cap)


def _build(QTAB, add_b1, add_b2):
    NCHUNK = sum(sum(r) for r in QTAB)
    TOT = NCHUNK * 128
    CB = [0] * NW  # first chunk index of each window
    for w in range(1, NW):
        CB[w] = CB[w - 1] + sum(QTAB[w - 1])

    nc = bacc.Bacc(num_swdge_queues=4, dynamic_dma_scratch_size=32768)
    GQ = [0]  # global gather counter; keeps Tile's 8-lane DMASW sem
    # rotation and the 4 HW SWDGE queues in a consistent pairing
    dp = nc.declare_dram_parameter
    # per-device inputs (xT is rolled per device)
    xT_d = dp("xT", [FIN, N], BF16, isOutput=False)
    srcidx_d = dp("srcidx", [128, TOT // 16], I16, isOutput=False)
    srcidx2_d = dp("srcidx2", [128, TOT // 16], I16, isOutput=False)
    dstloc_d = dp("dstloc", [128, NCHUNK], BF16, isOutput=False)
    dstrep_d = dp("dstrep", [128, TOT], BF16, isOutput=False)
    # shared inputs
    W1_d = dp("W1aug", [FIN, D1 + 16], BF16, isOutput=False)
    W2_d = dp("W2aug", [D1, C2 + 2], BF16, isOutput=False)
    iota_d = dp("iota32", [128, QMAX5 * 32], BF16, isOutput=False)
    iotac_d = dp("iota_col", [128, 1], F32, isOutput=False)
    ident_d = dp("ident", [128, 128], BF16, isOutput=False)
    b1_d = dp("b1bc", [128, D1], F32, isOutput=False)
    b2_d = dp("b2bc", [128, C2], F32, isOutput=False)
    # output
    out_d = dp("out", [NLOC, C2], F32, isOutput=True)
    # internal DRAM
    hcat = nc.dram_tensor("hcat", [N, ROW1], FP8)
    h2loc = nc.dram_tensor("h2loc", [NLOC, ROW2], BF16)
    h2all = nc.dram_tensor("h2all", [N, ROW2], BF16, addr_space="Shared")

    with tile.TileContext(nc) as tc:
        with tc.tile_pool(name="const", bufs=1) as constp:
            iota32_t = constp.tile([128, QMAX5 * 32], BF16)
            nc.sync.dma_start(iota32_t[:], iota_d[:])
            iotac_t = constp.tile([128, 1], F32)
            nc.sync.dma_start(iotac_t[:], iotac_d[:])
            ident_t = constp.tile([128, 128], BF16)
            nc.sync.dma_start(ident_t[:], ident_d[:])
            b1_t = b2_t = None
            if add_b1:
                b1_t = constp.tile([128, D1], F32)
                nc.sync.dma_start(b1_t[:], b1_d[:])
            if add_b2:
                b2_t = constp.tile([128, C2], F32)
                nc.sync.dma_start(b2_t[:], b2_d[:])
            srcidx_t = constp.tile([128, TOT // 16], I16)
            nc.sync.dma_start(srcidx_t[:], srcidx_d[:])
            srcidx2_t = constp.tile([128, TOT // 16], I16)
            nc.sync.dma_start(srcidx2_t[:], srcidx2_d[:])
            dstloc_t = constp.tile([128, NCHUNK], BF16)
            nc.sync.dma_start(dstloc_t[:], dstloc_d[:])
            dstrep_t = constp.tile([128, TOT], BF16)
            nc.sync.dma_start(dstrep_t[:], dstrep_d[:])
            W2_t = constp.tile([128, 8, C2 + 2], BF16)
            nc.sync.dma_start(
                W2_t[:], W2_d[:].rearrange("(k p) f -> p k f", p=128)
            )
            h1T_t = constp.tile([128, 8, NW * 128], BF16)
            sdw_t = constp.tile([128, NW, 8], BF16)
            ssw_t = constp.tile([128, NW, 8], BF16)
            hloc_t = constp.tile([128, NW, D1], BF16)
            h2loc_t = constp.tile([128, NW, C2], BF16)
            s2dcol_t = constp.tile([128, NW], BF16)
            nc.vector.memset(s2dcol_t[:], 0.0)
            s2scol_t = constp.tile([128, NW], BF16)
            nc.vector.memset(s2scol_t[:], 0.0)
            nc.vector.memset(h2loc_t[:], 0.0)

            # ---------------- Phase A: replicated h = x @ W1aug (rolled order)
            with (
                tc.tile_pool(name="gemmA", bufs=1) as gA,
                tc.tile_pool(name="outA", bufs=3) as oA,
                tc.tile_pool(name="psA", bufs=3, space="PSUM") as psA_p,
                tc.tile_pool(name="psAs", bufs=2, space="PSUM") as psAs_p,
            ):
                W1_t = gA.tile([128, 2, D1 + 16], BF16)
                nc.sync.dma_start(
                    W1_t[:], W1_d[:].rearrange("(k p) f -> p k f", p=128)
                )
                xT_t = gA.tile([128, 2, N], BF16)
                xTr = xT_d[:].rearrange("(k p) n -> p k n", p=128)
                for c0 in range(0, N, 2048):
                    c1 = min(N, c0 + 2048)
                    nc.sync.dma_start(
                        xT_t[:, :, c0:c1], xTr[:, :, c0:c1]
                    )
                for t in range(NT):
                    tl = min(128, N - t * 128)
                    ps = psA_p.tile([128, 1024], F32, tag="psA")
                    pss = psAs_p.tile([128, 16], F32, tag="psAs")
                    for k in range(2):
                        lhsT = xT_t[:, k, t * 128 : t * 128 + tl]
                        nc.tensor.matmul(
                            ps[:tl, 0:512], lhsT, W1_t[:, k, 0:512],
                            start=(k == 0), stop=(k == 1),
                        )
                        nc.tensor.matmul(
                            ps[:tl, 512:1024], lhsT, W1_t[:, k, 512:1024],
                            start=(k == 0), stop=(k == 1),
                        )
                        nc.tensor.matmul(
                            pss[:tl, 0:16], lhsT, W1_t[:, k, 1024:1040],
                            start=(k == 0), stop=(k == 1),
                        )
                    hc = oA.tile([128, ROW1], FP8, tag="hc")
                    nc.vector.tensor_copy(hc[:tl, 0:512], ps[:tl, 0:512])
                    nc.scalar.activation(
                        hc[:tl, 512:1024], ps[:tl, 512:1024], ACTF.Copy
                    )
                    nc.vector.tensor_copy(
                        hc[:tl].bitcast(BF16)[:, 512:520], pss[:tl, 0:8]
                    )
                    if t < NW:
                        nc.vector.tensor_copy(sdw_t[:, t, :], pss[:, 8:16])
                        nc.vector.tensor_copy(ssw_t[:, t, :], pss[:, 0:8])
                        nc.vector.tensor_copy(
                            hloc_t[:, t, 0:512], ps[:, 0:512]
                        )
                        nc.scalar.activation(
                            hloc_t[:, t, 512:1024], ps[:, 512:1024], ACTF.Copy
                        )
                    # only bytes 0:1040 of each row are ever read; the pad
                    # tail is left unwritten (gathered but unused)
                    nc.sync.dma_start(
                        hcat[t * 128 : t * 128 + tl, 0:1040], hc[:tl, 0:1040]
                    )

            # ---------------- Phase B+C: layer-1 aggregation + layer-2 GEMM
            # global chunk metadata: (window, quarter, first/last-in-quarter)
            CHMETA = []
            for w in range(NW):
                for q in range(4):
                    for c in range(QTAB[w][q]):
                        CHMETA.append((w, q, c == 0, c == QTAB[w][q] - 1))

            def _bc_group(
                kg0, kg1, gq, eB, hp, wB, es_p, psws, dens, on_done=None
            ):
                    gl = kg1 - kg0
                    hg = eB.tile([128, QMAX5, ROW1], FP8, tag="hg")
                    nc.gpsimd.dma_gather(
                        out_ap=hg[:, 0:gl, :], in_ap=hcat[:, :],
                        idxs_ap=srcidx_t[:, kg0 * 8 : kg1 * 8],
                        num_idxs=gl * 128,
                        num_idxs_reg=gl * 128,
                        elem_size=ROW1, single_packet=False,
                        queue_num=GQ[0] % 4,
                    )
                    GQ[0] += 1
                    # dst-major one-hot + per-edge dst scores
                    Ssb = wB.tile([128, QMAX5 * 128], BF16, tag="Ssb")
                    nc.vector.tensor_scalar(
                        Ssb[:, 0 : gl * 128],
                        dstrep_t[:, kg0 * 128 : kg1 * 128],
                        iotac_t[:, 0:1], None, ALU.is_equal,
                    )
                    esp = es_p.tile([128, 512], F32, tag="esp")
                    for j in range(gl):
                        wk = CHMETA[kg0 + j][0]
                        nc.tensor.matmul(
                            esp[:, j * 8 : j * 8 + 8],
                            Ssb[:, j * 128 : (j + 1) * 128],
                            sdw_t[:, wk, :],
                            start=(j == 0), stop=(j == gl - 1),
                        )
                    # p = exp(leaky_relu(ssrc + sdst))  [128, gl, 8]
                    pt = eB.tile([128, QMAX5, 8], F32, tag="pt")
                    nc.vector.tensor_tensor(
                        pt[:, 0:gl],
                        hg[:, 0:gl].bitcast(BF16)[:, :, 512:520],
                        esp[:, 0 : gl * 8].rearrange("e (c h) -> e c h", h=8),
                        ALU.add,
                    )
                    lr = eB.tile([128, QMAX5, 8], F32, tag="lr")
                    nc.vector.scalar_tensor_tensor(
                        lr[:, 0:gl], pt[:, 0:gl], SLOPE, pt[:, 0:gl],
                        ALU.mult, ALU.max,
                    )
                    pb = eB.tile([128, QMAX5, 8], BF16, tag="pb")
                    nc.scalar.activation(pb[:, 0:gl], lr[:, 0:gl], ACTF.Exp)
                    # 32-wide e-major one-hot (quarter-relative dsts),
                    # then alpha-weighted per-head copies for the lhsT
                    stg32 = wB.tile([128, QMAX5, 32], BF16, tag="stg32")
                    nc.vector.tensor_tensor(
                        stg32[:, 0:gl],
                        iota32_t[:, 0 : gl * 32].rearrange(
                            "e (c i) -> e c i", i=32
                        ),
                        dstloc_t[:, kg0:kg1]
                        .unsqueeze(2)
                        .broadcast_to([128, gl, 32]),
                        ALU.is_equal,
                    )
                    stg8 = hp.tile([128, QMAX5, 8, 32], BF16, tag="stg8")
                    nc.vector.tensor_tensor(
                        stg8[:, 0:gl],
                        stg32[:, 0:gl]
                        .unsqueeze(2)
                        .broadcast_to([128, gl, 8, 32]),
                        pb[:, 0:gl]
                        .unsqueeze(3)
                        .broadcast_to([128, gl, 8, 32]),
                        ALU.mult,
                    )
                    # aggregation: alpha rides the one-hot lhsT; the raw
                    # gathered fp8 rows are the matmul rhs (mixed dtype).
                    # One start/stop per 2KB psum zero region (4 head blocks
                    # per bank): the bank's start marks the whole region
                    # pending-zero, later blocks overwrite on first touch.
                    for j in range(gl):
                        wk, q, fc, lc = CHMETA[kg0 + j]
                        psw, den = psws[wk], dens[wk]
                        nc.tensor.matmul(
                            den[32 * q : 32 * q + 32, 0:8],
                            stg32[:, j, :], pb[:, j, :],
                            start=fc, stop=lc,
                            tile_position=(0, 32 * q),
                        )
                        for h in range(8):
                            nc.tensor.matmul(
                                psw[
                                    32 * q : 32 * q + 32,
                                    h * 128 : (h + 1) * 128,
                                ],
                                stg8[:, j, h, :],
                                hg[:, j, h * 128 : (h + 1) * 128],
                                start=fc and h % 4 == 0,
                                stop=lc and h % 4 == 3,
                                tile_position=(0, 32 * q),
                            )
                        if on_done is not None:
                            on_done(kg0 + j)

            def _bc_epilogue(w, psw, den, wB, oC, den_p, psC_p):
                    wl = 128 if w < NW - 1 else LASTW
                    # window epilogue: exact self-loop term from local bf16
                    # activations, then h1 = elu(agg/denom + b1); h1T via PE
                    asw = wB.tile([128, 8], F32, tag="asw")
                    nc.vector.tensor_tensor(
                        asw[:], ssw_t[:, w, :], sdw_t[:, w, :], ALU.add
                    )
                    asl = wB.tile([128, 8], F32, tag="asl")
                    nc.vector.scalar_tensor_tensor(
                        asl[:], asw[:], SLOPE, asw[:], ALU.mult, ALU.max
                    )
                    ase = wB.tile([128, 8], F32, tag="ase")
                    nc.scalar.activation(ase[:], asl[:], ACTF.Exp)
                    dens = wB.tile([128, 8], F32, tag="dens")
                    nc.vector.tensor_tensor(dens[:], den[:], ase[:], ALU.add)
                    nc.vector.tensor_scalar(
                        dens[:], dens[:], _EPS, None, ALU.max
                    )
                    rec = wB.tile([128, 8], F32, tag="rec")
                    nc.vector.reciprocal(rec[:], dens[:])
                    h1r = wB.tile([128, D1], F32, tag="h1r")
                    nc.vector.tensor_tensor(
                        h1r[:].rearrange("e (h c) -> e h c", c=C1),
                        hloc_t[:, w, :].rearrange("e (h c) -> e h c", c=C1),
                        ase[:].unsqueeze(2).broadcast_to([128, 8, C1]),
                        ALU.mult,
                    )
                    nc.vector.tensor_tensor(h1r[:], h1r[:], psw[:], ALU.add)
                    for half in range(2):
                        o = 512 * half
                        nc.vector.tensor_tensor(
                            h1r[:, o : o + 512].rearrange(
                                "e (h c) -> e h c", c=C1
                            ),
                            h1r[:, o : o + 512].rearrange(
                                "e (h c) -> e h c", c=C1
                            ),
                            rec[:, 4 * half : 4 * half + 4]
                            .unsqueeze(2)
                            .broadcast_to([128, 4, C1]),
                            ALU.mult,
                        )
                    if add_b1:
                        nc.vector.tensor_tensor(
                            h1r[:], h1r[:], b1_t[:], ALU.add
                        )
                    etmp = wB.tile([128, D1], F32, tag="etmp")
                    nc.scalar.activation(etmp[:], h1r[:], ACTF.Exp)
                    nc.vector.tensor_scalar(
                        etmp[:], etmp[:], 1.0, 0.0, ALU.subtract, ALU.min
                    )
                    nc.vector.tensor_scalar(
                        h1r[:], h1r[:], 0.0, None, ALU.max
                    )
                    h1b = wB.tile([128, D1], BF16, tag="h1b")
                    nc.vector.tensor_tensor(h1b[:], h1r[:], etmp[:], ALU.add)
                    for j in range(8):
                        tp = den_p.tile([128, 1024], BF16, tag="tp")
                        nc.tensor.transpose(
                            tp[:, 0:128], h1b[:, j * 128 : (j + 1) * 128],
                            ident_t[:],
                        )
                        nc.scalar.activation(
                            h1T_t[:, j, w * 128 : w * 128 + wl],
                            tp[:, 0:wl], ACTF.Copy,
                        )
                    # fused layer-2 GEMM for this window
                    ps2 = psC_p.tile([128, 512], F32, tag="ps2")
                    for k in range(8):
                        nc.tensor.matmul(
                            ps2[:wl, 0 : C2 + 2],
                            h1T_t[:, k, w * 128 : w * 128 + wl],
                            W2_t[:, k, :],
                            start=(k == 0), stop=(k == 7),
                        )
                    h2t = oC.tile([128, ROW2], BF16, tag="h2t")
                    nc.vector.tensor_copy(h2t[:wl, 0:128], ps2[:wl, 0:128])
                    nc.vector.tensor_copy(
                        h2t[:wl].bitcast(F32)[:, 64:65], ps2[:wl, 128:129]
                    )
                    nc.vector.tensor_copy(
                        s2dcol_t[:wl, w : w + 1], ps2[:wl, 129:130]
                    )
                    nc.vector.tensor_copy(
                        s2scol_t[:wl, w : w + 1], ps2[:wl, 128:129]
                    )
                    nc.vector.tensor_copy(
                        h2loc_t[:wl, w, :], ps2[:wl, 0:128]
                    )
                    nc.sync.dma_start(
                        h2loc[w * 128 : w * 128 + wl, 0:130],
                        h2t[:wl, 0:130],
                    )

            def _bc_scope(ws):
                with (
                    tc.tile_pool(name="edgeB", bufs=5) as eB,
                    tc.tile_pool(name="hpB", bufs=3) as hp,
                    tc.tile_pool(name="winB", bufs=3) as wB,
                    tc.tile_pool(name="outC", bufs=2) as oC,
                    tc.tile_pool(name="psw", bufs=2, space="PSUM") as psw_p,
                    tc.tile_pool(name="den", bufs=1, space="PSUM") as den_p,
                    tc.tile_pool(name="esp", bufs=1, space="PSUM") as es_p,
                    tc.tile_pool(name="psC", bufs=1, space="PSUM") as psC_p,
                ):
                    klo = CB[ws[0]]
                    khi = CB[ws[-1]] + sum(QTAB[ws[-1]])
                    lastk = {w: CB[w] + sum(QTAB[w]) - 1 for w in ws}
                    # one persistent den bank, even/odd windows use col
                    # halves (region-precise dep tracking makes this a
                    # manual double-buffer)
                    den_all = den_p.tile([128, 512], F32, name="den_all")
                    psws, dens = {}, {}
                    gq = 0
                    for kg0 in range(klo, khi, QMAX5):
                        kg1 = min(kg0 + QMAX5, khi)
                        for j in range(kg0, kg1):
                            wk = CHMETA[j][0]
                            if wk not in psws:
                                psws[wk] = psw_p.tile(
                                    [128, 1024], F32, tag="psw",
                                    name=f"psw{wk}",
                                )
                                o8 = (wk % 2) * 8
                                dens[wk] = den_all[:, o8 : o8 + 8]
                        def _done(j):
                            wk = CHMETA[j][0]
                            if j == lastk[wk]:
                                _bc_epilogue(
                                    wk, psws[wk], dens[wk], wB, oC,
                                    den_p, psC_p,
                                )

                        _bc_group(
                            kg0, kg1, gq, eB, hp, wB, es_p, psws, dens,
                            on_done=_done,
                        )
                        gq += 1

            if AGSPLIT:
                _bc_scope(range(5))
                nc.gpsimd.collective_compute(
                    "AllGather",
                    ALU.bypass,
                    ins=[h2loc[0:SPLIT, :]],
                    outs=[h2all[0 : NDEV * SPLIT, :]],
                    replica_groups=[list(range(NDEV))],
                )
                _bc_scope(range(5, NW))
                nc.gpsimd.collective_compute(
                    "AllGather",
                    ALU.bypass,
                    ins=[h2loc[SPLIT:NLOC, :]],
                    outs=[h2all[NDEV * SPLIT : N, :]],
                    replica_groups=[list(range(NDEV))],
                )
            else:
                _bc_scope(range(NW))
                nc.gpsimd.collective_compute(
                    "AllGather",
                    ALU.bypass,
                    ins=[h2loc[:]],
                    outs=[h2all[:]],
                    replica_groups=[list(range(NDEV))],
                )

            # ---------------- Phase D: layer-2 edge aggregation
            with (
                tc.tile_pool(name="edgeD", bufs=4) as eD,
                tc.tile_pool(name="winD", bufs=3) as wD,
                tc.tile_pool(name="psw2", bufs=2, space="PSUM") as psw2_p,
                tc.tile_pool(name="den2", bufs=1, space="PSUM") as den2_p,
                tc.tile_pool(name="esp2", bufs=1, space="PSUM") as es2_p,
            ):
                def _d_group(kg0, kg1, gq, psw2s, den2s, on_done=None):
                    gl = kg1 - kg0
                    g2 = eD.tile([128, QMAX5, ROW2], BF16, tag="g2")
                    nc.gpsimd.dma_gather(
                        out_ap=g2[:, 0:gl, :], in_ap=h2all[:, :],
                        idxs_ap=srcidx2_t[:, kg0 * 8 : kg1 * 8],
                        num_idxs=gl * 128,
                        num_idxs_reg=gl * 128,
                        elem_size=ROW2, single_packet=False,
                        queue_num=GQ[0] % 4,
                    )
                    GQ[0] += 1
                    Ssb2 = wD.tile([128, QMAX5 * 128], BF16, tag="Ssb2")
                    nc.vector.tensor_scalar(
                        Ssb2[:, 0 : gl * 128],
                        dstrep_t[:, kg0 * 128 : kg1 * 128],
                        iotac_t[:, 0:1], None, ALU.is_equal,
                    )
                    esp2 = es2_p.tile([128, 512], F32, tag="esp2")
                    for j in range(gl):
                        wk = CHMETA[kg0 + j][0]
                        nc.tensor.matmul(
                            esp2[:, j : j + 1],
                            Ssb2[:, j * 128 : (j + 1) * 128],
                            s2dcol_t[:, wk : wk + 1],
                            start=(j == 0), stop=(j == gl - 1),
                        )
                    pt2 = eD.tile([128, QMAX5, 1], F32, tag="pt2")
                    nc.vector.tensor_tensor(
                        pt2[:, 0:gl],
                        g2[:, 0:gl].bitcast(F32)[:, :, 64:65],
                        esp2[:, 0:gl].unsqueeze(2),
                        ALU.add,
                    )
                    lr2 = eD.tile([128, QMAX5, 1], F32, tag="lr2")
                    nc.vector.scalar_tensor_tensor(
                        lr2[:, 0:gl], pt2[:, 0:gl], SLOPE, pt2[:, 0:gl],
                        ALU.mult, ALU.max,
                    )
                    pb2 = eD.tile([128, QMAX5, 1], BF16, tag="pb2")
                    nc.scalar.activation(pb2[:, 0:gl], lr2[:, 0:gl], ACTF.Exp)
                    stg32d = wD.tile([128, QMAX5, 32], BF16, tag="stg32d")
                    nc.vector.tensor_tensor(
                        stg32d[:, 0:gl],
                        iota32_t[:, 0 : gl * 32].rearrange(
                            "e (c i) -> e c i", i=32
                        ),
                        dstloc_t[:, kg0:kg1]
                        .unsqueeze(2)
                        .broadcast_to([128, gl, 32]),
                        ALU.is_equal,
                    )
                    stg1 = wD.tile([128, QMAX5, 32], BF16, tag="stg1")
                    nc.vector.tensor_tensor(
                        stg1[:, 0:gl],
                        stg32d[:, 0:gl],
                        pb2[:, 0:gl].broadcast_to([128, gl, 32]),
                        ALU.mult,
                    )
                    for j in range(gl):
                        wk, q, fc, lc = CHMETA[kg0 + j]
                        nc.tensor.matmul(
                            den2s[wk][32 * q : 32 * q + 32, 0:1],
                            stg32d[:, j, :], pb2[:, j, :],
                            start=fc, stop=lc,
                            tile_position=(0, 32 * q),
                        )
                        nc.tensor.matmul(
                            psw2s[wk][32 * q : 32 * q + 32, 0:128],
                            stg1[:, j, :], g2[:, j, 0:128],
                            start=fc, stop=lc,
                            tile_position=(0, 32 * q),
                        )
                        if on_done is not None:
                            on_done(kg0 + j)

                def _d_epilogue(w, psw2, den2):
                    wl = 128 if w < NW - 1 else LASTW
                    as2 = wD.tile([128, 1], F32, tag="as2")
                    nc.vector.tensor_tensor(
                        as2[:], s2scol_t[:, w : w + 1],
                        s2dcol_t[:, w : w + 1], ALU.add,
                    )
                    as2l = wD.tile([128, 1], F32, tag="as2l")
                    nc.vector.scalar_tensor_tensor(
                        as2l[:], as2[:], SLOPE, as2[:], ALU.mult, ALU.max
                    )
                    as2e = wD.tile([128, 1], F32, tag="as2e")
                    nc.scalar.activation(as2e[:], as2l[:], ACTF.Exp)
                    dens2 = wD.tile([128, 1], F32, tag="dens2")
                    nc.vector.tensor_tensor(
                        dens2[:], den2[:, 0:1], as2e[:], ALU.add
                    )
                    nc.vector.tensor_scalar(
                        dens2[:], dens2[:], _EPS, None, ALU.max
                    )
                    rec2 = wD.tile([128, 1], F32, tag="rec2")
                    nc.vector.reciprocal(rec2[:], dens2[:])
                    ot = wD.tile([128, C2], F32, tag="ot")
                    nc.vector.tensor_scalar(
                        ot[:], h2loc_t[:, w, :], as2e[:, 0:1], None, ALU.mult
                    )
                    nc.vector.tensor_tensor(
                        ot[:], ot[:], psw2[:, 0:128], ALU.add
                    )
                    nc.vector.tensor_scalar(
                        ot[:], ot[:], rec2[:, 0:1], None, ALU.mult
                    )
                    if add_b2:
                        nc.vector.tensor_tensor(ot[:], ot[:], b2_t[:], ALU.add)
                    nc.sync.dma_start(
                        out_d[w * 128 : w * 128 + wl, :], ot[:wl, :]
                    )

                lastk = {
                    w: CB[w] + sum(QTAB[w]) - 1 for w in range(NW)
                }
                den2_all = den2_p.tile([128, 512], F32, name="den2_all")
                psw2s, den2s = {}, {}
                gq = 0
                for kg0 in range(0, NCHUNK, QMAX5):
                    kg1 = min(kg0 + QMAX5, NCHUNK)
                    for j in range(kg0, kg1):
                        wk = CHMETA[j][0]
                        if wk not in psw2s:
                            psw2s[wk] = psw2_p.tile(
                                [128, 512], F32, tag="psw2", name=f"psw2{wk}"
                            )
                            o2 = wk % 2
                            den2s[wk] = den2_all[:, o2 : o2 + 1]
                    def _done(j):
                        wk = CHMETA[j][0]
                        if j == lastk[wk]:
                            _d_epilogue(wk, psw2s[wk], den2s[wk])

                    _d_group(kg0, kg1, gq, psw2s, den2s, on_done=_done)
                    gq += 1

    nc.finalize()
    return nc


# ------------------------------------------------------------------ entrypoint

TRACE = [False]
LAST = [None]


def kernel(x, edge_index, W1, a_src1, a_dst1, b1, W2, a_src2, a_dst2, b2):
    x = np.asarray(x, np.float32)
    W1 = np.asarray(W1, np.float32)
    W2 = np.asarray(W2, np.float32)
    a_src1 = np.asarray(a_src1, np.float32)
    a_dst1 = np.asarray(a_dst1, np.float32)
    a_src2 = np.asarray(a_src2, np.float32)
    a_dst2 = np.asarray(a_dst2, np.float32)
    b1 = np.asarray(b1, np.float32)
    b2 = np.asarray(b2, np.float32)
    ei = np.asarray(edge_index)

    devs, QTAB, order_ids = _prep_edges(ei)

    # fold attention projections into the GEMM weights
    A1 = np.zeros((D1, 16), np.float32)
    for h in range(H1):
        A1[h * C1 : (h + 1) * C1, h] = a_src1[h]
        A1[h * C1 : (h + 1) * C1, 8 + h] = a_dst1[h]

    W1aug = np.concatenate([W1, W1 @ A1], axis=1)
    W2aug = np.concatenate(
        [W2, W2 @ a_src2[0][:, None], W2 @ a_dst2[0][:, None]], 1
    )
    add_b1 = bool(np.any(b1 != 0))
    add_b2 = bool(np.any(b2 != 0))

    key = (tuple(tuple(r) for r in QTAB), add_b1, add_b2, AGSPLIT)
    if key not in _CACHE:
        _CACHE[key] = _build(QTAB, add_b1, add_b2)
    nc = _CACHE[key]

    xT = x.T
    shared = {
        "W1aug": _bf(W1aug),
        "W2aug": _bf(W2aug),
        "iota32": _bf(
            np.tile(np.arange(32, dtype=np.float32)[None, :], (128, QMAX5))
        ),
        "iota_col": np.arange(128, dtype=np.float32)[:, None].copy(),
        "ident": _bf(np.eye(128, dtype=np.float32)),
        "b1bc": np.ascontiguousarray(np.tile(b1[None, :], (128, 1))),
        "b2bc": np.ascontiguousarray(np.tile(b2[None, :], (128, 1))),
    }
    in_maps = []
    for d in range(NDEV):
        m = {**shared, **devs[d]}
        rolled = np.concatenate(
            [order_ids[d * NLOC :], order_ids[: d * NLOC]]
        )
        m["xT"] = _bf(xT[:, rolled])
        in_maps.append(m)

    if os.environ.get("KSIM"):
        from concourse.bass_interp import MultiCoreSim

        sim = MultiCoreSim(
            nc,
            num_cores=NDEV,
            num_workers=int(os.environ.get("KSIM_WORKERS", "8")),
            require_finite=False,
            require_nnan=False,
        )
        for d in range(NDEV):
            cs = sim.cores[d]
            for k2, v in in_maps[d].items():
                cs.tensor(k2)[:] = v
        sim.simulate(check_with_hw=False)
        cat = np.concatenate(
            [np.array(sim.cores[d].tensor("out")) for d in range(NDEV)], axis=0
        )
        LAST[0] = None
        out = np.empty_like(cat)
        out[order_ids] = cat
        return out.astype(np.float32)

    res = run_bass_kernel_spmd(nc, in_maps, list(range(NDEV)), trace=TRACE[0])
    LAST[0] = res
    cat = np.concatenate([res.results[d]["out"] for d in range(NDEV)], axis=0)
    out = np.empty_like(cat)
    out[order_ids] = cat
    return out.astype(np.float32)

